# revision 6
# baseline (speedup 1.0000x reference)
"""KoLeo loss kernel v2 for Trainium2 (8 NeuronCores, Bass/Tile).

Math: reference normalizes rows of student_output [8192, 384], finds each
row's nearest neighbor by cosine similarity (self masked), and returns
  loss = -mean(log(||x_i - x_nn|| + eps)).
For unit vectors ||x_i - x_j||^2 = 2 - 2*dot(x_i, x_j), so only the max
off-diagonal dot per row is needed.

Design (see git of this file for the v1 on-device-normalize variant):
- Normalize + transpose + fp8 quantize on HOST (linear-time prep). The
  device receives x^T pre-normalized, scaled by 16, as fp8e4m3 in four
  96-row contraction subtiles.
- Matmuls run in fp8 DoubleRow perf mode: 192 contraction rows per
  512-column pass -> 2 instructions cover D=384 per PSUM chunk (bf16
  needs 3), and fp8 halves the input DMA. The PE streams 1 column/cycle,
  so the per-core floor is 8 mt * 8192 cols * 2 passes = 131072 cycles
  (~55us at 2.4 GHz); dummy warm-up matmuls during the DMA ramp keep the
  PE out of its low-frequency pstate for the real stream.
- Row-max reduce of the [128, 8192] similarity block is split between
  DVE (reduce_max direct from PSUM) and ACT (exp-sum accumulator) units
  using the log-sum-exp identity: for beta=384 and this problem's
  ~0.012 typical top-2 similarity gap, lse overestimates the row max by
  <1e-3, far inside the 2e-2 loss tolerance. ACT units need no DVE
  second stage, so both engines drain PSUM concurrently while the PE
  streams the next units (4 PSUM buffers of [128, 1024]).
- Input DMA configs are split across the sync (xqA) and vector (xqB)
  sequencers; outputs stream out per-mt on the idle gpsimd engine.

Per-core layout: core m gets x^T rolled by 1024*m columns; its stationary
block (own 1024 rows) is columns [0, 1024), pinning the self-match
diagonal of m-tile mt to columns [mt*128, mt*128+128) of unit g=0, which
is masked by adding -1024 * eye(128) to PSUM before the DVE reduce.
"""

import os
import numpy as np
import ml_dtypes

import concourse.bass as bass
import concourse.tile as tile
from concourse import bacc, mybir
from concourse.bass_utils import run_bass_kernel_spmd

F32 = mybir.dt.float32
FP16 = mybir.dt.float16
BF16 = mybir.dt.bfloat16
FP8 = mybir.dt.float8e4
AX = mybir.AxisListType
OP = mybir.AluOpType
AF = mybir.ActivationFunctionType
DR = mybir.MatmulPerfMode.DoubleRow

N, D = 8192, 384
P = 128
NCORES = 8
KSUB = 96              # contraction subtile rows (4 x 96 = 384)
MT = 8                 # stationary m-tiles of 128 rows
NG = 8                 # column units of 1024 per m-tile row block
NWARM = 5              # PE pstate warm-up matmuls during the DMA ramp
SCALE = 16.0           # host scale on normalized rows; dots scale 256
MASKVAL = -1024.0      # diag additive mask in scaled units
BETA = 384.0           # lse sharpness (in cosine units)
MTILDE = 0.26          # lse shift (approximate row max, cosine units)
# activation computes exp(scale*psum + bias) with psum = 256*cos:
ACT_SCALE = BETA / (SCALE * SCALE)        # 1.5
ACT_BIAS = -BETA * MTILDE                 # -99.84

# unit kind per (mt, g): True = DVE reduce_max, False = ACT exp-sum.
# g=0 carries the masked diagonal; the exp path would overflow on the
# unmasked self-dot, so g=0 stays DVE (mask applied on PSUM first).
KIND_DVE = [[g in (0, 1, 4, 5) for g in range(NG)] for _ in range(MT)]

_CACHE = {}


def _build_program():
    nc = bacc.Bacc("TRN2", target_bir_lowering=False, debug=False,
                   num_devices=NCORES)
    xq_in = nc.dram_tensor("xq", [4, KSUB, N], FP8, kind="ExternalInput").ap()
    negid_in = nc.dram_tensor("negid", [P, P], F32, kind="ExternalInput").ap()
    out_dram = nc.dram_tensor("out", [P, MT * 2 * NG], F32,
                              kind="ExternalOutput").ap()

    with tile.TileContext(nc) as tc:
        with (
            tc.tile_pool(name="consts", bufs=1) as const_pool,
            tc.tile_pool(name="xq", bufs=1) as xq_pool,
            tc.tile_pool(name="out", bufs=1) as out_pool,
            tc.tile_pool(name="junk", bufs=4) as junk_pool,
            tc.tile_pool(name="psum", bufs=4, space="PSUM") as psum_pool,
        ):
            negid = const_pool.tile([P, P], F32)
            bias_t = const_pool.tile([P, 1], F32, name="bias_t")

            xqA = xq_pool.tile([KSUB, 2, N], FP8, name="xqA")
            xqB = xq_pool.tile([KSUB, 2, N], FP8, name="xqB")
            # column-chunk loads, finest chunks first so the first matmuls
            # start early; xqA configs on sync, xqB on scalar
            chunks = [(0, 1024), (1024, 2048), (2048, 4096), (4096, 6144),
                      (6144, 8192)]
            with tc.high_priority():
                for c0, c1 in chunks:
                    cs = slice(c0, c1)
                    nc.sync.dma_start(xqA[:, 0, cs], xq_in[0, :, cs])
                    nc.sync.dma_start(xqA[:, 1, cs], xq_in[1, :, cs])
                    nc.scalar.dma_start(xqB[:, 0, cs], xq_in[2, :, cs])
                    nc.scalar.dma_start(xqB[:, 1, cs], xq_in[3, :, cs])
                nc.scalar.dma_start(negid, negid_in)
                # PE warm-up source + junk DR matmuls while inputs load:
                # keeps the PE out of its low-frequency pstate
                wsrc = const_pool.tile([KSUB, 2, 640], FP8, name="wsrc")
                nc.vector.memset(wsrc, 0.0)
                wps = psum_pool.tile([P, 1024], F32, tag="ps", name="wps")
                for i in range(NWARM):
                    nc.tensor.matmul(wps[:, 0:512], wsrc[:, :, 0:128],
                                     wsrc[:, :, 128:640],
                                     start=True, stop=True, perf_mode=DR)
                nc.gpsimd.memset(bias_t, ACT_BIAS)
                # dummy exp to pull ACT_TABLE_LOAD into the DMA ramp
                warm = const_pool.tile([P, 1], F32, name="warm")
                nc.scalar.activation(warm, bias_t, AF.Exp)

            # per-mt output tile: cols [0:NG] = DVE max, [NG:2*NG] = ACT sums
            outs_t = []
            for mt in range(MT):
                ot = out_pool.tile([P, 2 * NG], F32, name=f"out{mt}")
                nc.gpsimd.memset(ot, 0.0)
                outs_t.append(ot)

            def consume(ps, mt, g):
                if g == 0:
                    o = mt * P
                    nc.vector.tensor_add(ps[:, o:o + P], ps[:, o:o + P],
                                         negid)
                if KIND_DVE[mt][g]:
                    nc.vector.reduce_max(outs_t[mt][:, g:g + 1], ps,
                                         axis=AX.X)
                else:
                    jk = junk_pool.tile([P, 1024], BF16, tag="jk")
                    nc.scalar.activation(jk, ps, AF.Exp, bias=bias_t,
                                         scale=ACT_SCALE,
                                         accum_out=outs_t[mt][:, NG + g:
                                                              NG + g + 1])

            for mt in range(MT):
                ms = slice(mt * P, (mt + 1) * P)
                for w in range(2):
                    gs = range(4 * w, 4 * w + 4)
                    pss = [psum_pool.tile([P, 1024], F32, tag="ps",
                                          name=f"ps{mt}_{g}")
                           for g in gs]
                    for xt, startf in ((xqA, True), (xqB, False)):
                        for ps, g in zip(pss, gs):
                            for j in range(2):
                                c0 = g * 1024 + j * 512
                                nc.tensor.matmul(
                                    ps[:, j * 512:(j + 1) * 512],
                                    xt[:, :, ms],
                                    xt[:, :, c0:c0 + 512],
                                    start=startf, stop=not startf,
                                    perf_mode=DR)
                    for ps, g in zip(pss, gs):
                        consume(ps, mt, g)
                mo = slice(mt * 2 * NG, (mt + 1) * 2 * NG)
                nc.gpsimd.dma_start(out_dram[:, mo], outs_t[mt])

    nc.compile()
    return nc


def _get_program():
    if "nc" not in _CACHE:
        _CACHE["nc"] = _build_program()
    return _CACHE["nc"]


def _quantize(student_output: np.ndarray) -> np.ndarray:
    x = np.asarray(student_output, dtype=np.float64)
    assert x.shape == (N, D)
    norm = np.linalg.norm(x, axis=1, keepdims=True)
    xn = (x / np.maximum(norm, 1e-8)) * SCALE
    return xn.astype(ml_dtypes.float8_e4m3)


def _make_in_maps(student_output: np.ndarray):
    xq = _quantize(student_output)
    negid = (MASKVAL * np.eye(P)).astype(np.float32)
    in_maps = []
    for m in range(NCORES):
        xr = np.roll(xq, -1024 * m, axis=0)
        xqT = np.ascontiguousarray(xr.T).reshape(4, KSUB, N)
        in_maps.append({"xq": xqT, "negid": negid})
    return in_maps


def _combine(results) -> np.float32:
    md = np.empty(N, dtype=np.float64)
    s2 = SCALE * SCALE
    with np.errstate(divide="ignore"):
        for m in range(NCORES):
            out = np.asarray(results[m]["out"], dtype=np.float64)
            for mt in range(MT):
                base = mt * 2 * NG
                dcols = [base + g for g in range(NG) if KIND_DVE[mt][g]]
                acols = [base + NG + g for g in range(NG)
                         if not KIND_DVE[mt][g]]
                dmax = out[:, dcols].max(axis=1) / s2
                cand = dmax
                if acols:
                    stot = out[:, acols].sum(axis=1)
                    lse = MTILDE + np.log(stot) / BETA
                    cand = np.maximum(dmax, lse)
                md[m * 1024 + mt * P:m * 1024 + (mt + 1) * P] = cand
    d2 = np.maximum(2.0 - 2.0 * md, 0.0)
    d = np.sqrt(d2)
    loss = -np.mean(np.log(d + 1e-8))
    return np.float32(loss)


def run(student_output: np.ndarray, trace: bool = False):
    nc = _get_program()
    in_maps = _make_in_maps(student_output)
    res = run_bass_kernel_spmd(nc, in_maps, core_ids=list(range(NCORES)),
                               trace=trace)
    return _combine(res.results), res


def kernel(student_output: np.ndarray) -> np.ndarray:
    out, _ = run(student_output,
                 trace=bool(int(os.environ.get("KOLEO_TRACE", "0"))))
    return out
